# revision 39
# baseline (speedup 1.0000x reference)
"""Trainium2 Bass kernel for a pre-LN transformer block (B=4, T=2048, E=1024, H=16).

Sharding: 8 cores = 4 batches x 2 token-halves. Each core receives its batch's
full 2048 tokens (rolled so its own 1024 query tokens come first), computes
K/V for all 2048 tokens (redundantly with its pair core) and everything else
(Q, attention, proj, FFN) only for its own 1024 tokens. Zero cross-core
communication; host reassembles the output.

Pipeline structure (single interleaved emission so the softmax exp — the
ScalarE-bound critical path, ~33.5M elements/core — starts ~30us in):
  ramp:      LN1 group g -> K(mf0/1,g), Q(mf0/1,g), V(st in g)   [per group]
  attention: scores/exp/AV(fp8 DoubleRow) steps, with K/Q chunks for
             heads 4..15 interleaved 1-per-2-steps under the ACT-bound window
  tail:      proj(mt) -> LN2(mt) pipelined; FFN1 in token-halves; FFN2
             (nbh0, mt0-3) interleaved with FFN1 second half.

LayerNorm gains/biases fold into matmul weights host-side. K bias is dropped
entirely (softmax is invariant to per-query score shifts). Softmax denominator
reciprocal runs on [128,8] (DRAM-bounced) instead of [1,1024]. All attention
operands are fp8e4 (validated ~1e-3 added error); FFN stays bf16 (fp8 fails
the 2e-2 budget).
"""

import numpy as np
import ml_dtypes

BF = ml_dtypes.bfloat16
F8 = ml_dtypes.float8_e4m3

B, T, E, H, HS, FF = 4, 2048, 1024, 16, 64, 4096
TQ = T // 2          # own query tokens per core
NCORES = 8
EPS = 1e-5
NMT = T // 128       # 16 token tiles (full batch)
NMQ = TQ // 128      # 8 token tiles (own half)
NJE = E // 128       # 8 feature tiles of E
NJF = FF // 128      # 32 feature tiles of FF
NP = NMT // 2        # 8 key-tile pairs per head

_CACHE = {}
TRACE = False        # set by test harness to capture an NTFF profile
LAST_RESULTS = None  # BassKernelResults from the most recent run


def _build():
    import concourse.bacc as bacc
    import concourse.tile as tile
    from concourse import mybir
    from contextlib import ExitStack

    f32 = mybir.dt.float32
    bf16 = mybir.dt.bfloat16
    f8 = mybir.dt.float8e4
    DR = mybir.MatmulPerfMode.DoubleRow
    AF = mybir.ActivationFunctionType
    OP = mybir.AluOpType

    nc = bacc.Bacc("TRN2", target_bir_lowering=False, debug=False,
                   num_devices=NCORES)

    # ---- DRAM I/O ----
    x_d = nc.declare_dram_parameter("x", [T, E], bf16, isOutput=False)
    wq_d = nc.declare_dram_parameter("wq", [NJE, 128, E], f8, isOutput=False)
    wk_d = nc.declare_dram_parameter("wk", [NJE, 128, E], f8, isOutput=False)
    wv_d = nc.declare_dram_parameter("wv", [NJE, 128, E], f8, isOutput=False)
    wo_d = nc.declare_dram_parameter("wo", [NJE, 128, E], f8, isOutput=False)
    w1_d = nc.declare_dram_parameter("w1", [NJF, 128, E], bf16, isOutput=False)
    w2_d = nc.declare_dram_parameter("w2", [NJF, 128, E], bf16, isOutput=False)
    cq_d = nc.declare_dram_parameter("cq", [128, NJE], f32, isOutput=False)
    bob_d = nc.declare_dram_parameter("bob", [128, E], f32, isOutput=False)
    z1_d = nc.declare_dram_parameter("z1", [1, H * TQ], f8, isOutput=False)
    b2b_d = nc.declare_dram_parameter("b2b", [128, E], f32, isOutput=False)
    b1c_d = nc.declare_dram_parameter("b1c", [128, NJF], f32, isOutput=False)
    out_d = nc.declare_dram_parameter("out", [TQ, E], f32, isOutput=True)
    rbounce = nc.dram_tensor("rbounce", [H, TQ], f32)
    rbinv = nc.dram_tensor("rbinv", [H, 128, TQ // 128], f32)

    def layernorm(stats_pool, x_sb, out_bf, eps_sb):
        st = stats_pool.tile([128, 2, 6], f32, name="ln_st")
        nc.vector.bn_stats(out=st[:, 0, :], in_=x_sb[:, 0:512])
        nc.vector.bn_stats(out=st[:, 1, :], in_=x_sb[:, 512:1024])
        mv = stats_pool.tile([128, 2], f32, name="ln_mv")
        nc.vector.bn_aggr(out=mv[:], in_=st[:])
        rstd = stats_pool.tile([128, 1], f32, name="ln_rstd")
        nc.scalar.activation(out=rstd[:], in_=mv[:, 1:2], func=AF.Sqrt,
                             bias=eps_sb[:])
        nc.vector.reciprocal(out=rstd[:], in_=rstd[:])
        nmr = stats_pool.tile([128, 1], f32, name="ln_nmr")
        nc.vector.tensor_tensor(out=nmr[:], in0=mv[:, 0:1], in1=rstd[:],
                                op=OP.mult)
        nc.vector.tensor_scalar_mul(out=nmr[:], in0=nmr[:], scalar1=-1.0)
        nc.scalar.activation(out=out_bf[:], in_=x_sb[:], func=AF.Identity,
                             bias=nmr[:], scale=rstd[:])

    with tile.TileContext(nc) as tc:
        top = ExitStack()
        const = top.enter_context(tc.tile_pool(name="const", bufs=1, side="left"))
        eps_sb = const.tile([128, 1], f32)
        nc.vector.memset(eps_sb[:], EPS)
        cq_sb = const.tile([128, NJE], f32)
        nc.sync.dma_start(out=cq_sb[:], in_=cq_d[:])
        bob_sb = const.tile([128, E], f32)
        nc.sync.dma_start(out=bob_sb[:], in_=bob_d[:])

        # ---- long-lived left pools (open first so short-lived ones can
        # close underneath them in LIFO order) ----
        oT_es = ExitStack()
        oT = oT_es.enter_context(tc.tile_pool(name="oT", bufs=1, side="left")) \
            .tile([128, NJE, TQ], f8)                  # normalized attn out^T
        wop_es = ExitStack()
        wo_sb = wop_es.enter_context(
            tc.tile_pool(name="proj_w", bufs=1, side="left")) \
            .tile([128, NJE, E], f8)
        h2T_es = ExitStack()
        h2T = h2T_es.enter_context(
            tc.tile_pool(name="h2T", bufs=1, side="left")) \
            .tile([128, NMQ, NJE, 128], bf16)

        # ---- right pools ----
        xr_pool = top.enter_context(tc.tile_pool(name="xr", bufs=1, side="right"))
        xr_t = [xr_pool.tile([128, E], bf16, name=f"xr{i}") for i in range(NMQ)]
        xo_es = ExitStack()
        xown_pool = xo_es.enter_context(
            tc.tile_pool(name="x_own", bufs=1, side="right"))
        qkv_es = ExitStack()
        qkv_pool = qkv_es.enter_context(
            tc.tile_pool(name="qkvact", bufs=1, side="right"))
        # q^T zero-padded per head (head h on partitions (h%2)*64..+64, other
        # rows zero so the scores matmul contracts K=128 at full HAM clock)
        qT = qkv_pool.tile([128, H, TQ], f8)
        kT = qkv_pool.tile([128, NJE, T], f8)
        # v + ones col, fp8, paired key-tiles for DoubleRow AV matmuls
        v_aug = qkv_pool.tile([128, NP, 2, H, HS + 1], f8)
        wkq_es = ExitStack()
        wkq = wkq_es.enter_context(
            tc.tile_pool(name="w_kq", bufs=2, side="right"))
        wk_sb = wkq.tile([128, NJE, E], f8, name="wt")
        wq_sb = wkq.tile([128, NJE, E], f8, name="wt")
        wv_es = ExitStack()
        wv_sb = wv_es.enter_context(
            tc.tile_pool(name="w_v", bufs=1, side="right")) \
            .tile([128, NJE, E], f8)
        at_es = ExitStack()
        atp = at_es.enter_context(tc.tile_pool(name="att_t", bufs=3, side="right"))
        rp = at_es.enter_context(tc.tile_pool(name="att_r", bufs=2, side="right"))
        rqp = at_es.enter_context(tc.tile_pool(name="att_rq", bufs=2, side="right"))
        rbp = at_es.enter_context(tc.tile_pool(name="att_rb", bufs=2, side="right"))
        oup = at_es.enter_context(tc.tile_pool(name="att_ou", bufs=2, side="right"))

        # ---- short-lived left pools (h8 below the LN1 pools so LN1 pops first) ----
        h8_es = ExitStack()
        h8p = h8_es.enter_context(tc.tile_pool(name="h8", bufs=1, side="left"))
        h8 = [h8p.tile([128, NJE, 4, 128], f8, name=f"h8{g}") for g in range(4)]
        ln_es = ExitStack()
        xo_pool = ln_es.enter_context(tc.tile_pool(name="ln1x", bufs=1, side="left"))
        hTp = ln_es.enter_context(tc.tile_pool(name="hT", bufs=2, side="left"))
        stp = ln_es.enter_context(tc.tile_pool(name="ln1s", bufs=8, side="left"))
        hbp = ln_es.enter_context(tc.tile_pool(name="ln1h", bufs=3, side="left"))

        # ---- PSUM pools for LN1+QKV+attention ----
        psA_es = ExitStack()
        aps = psA_es.enter_context(tc.tile_pool(name="att_ps", bufs=2, space="PSUM"))
        ops = psA_es.enter_context(tc.tile_pool(name="att_po", bufs=3, space="PSUM"))
        qkps = psA_es.enter_context(tc.tile_pool(name="qkv_ps", bufs=1, space="PSUM"))

        # ---- input and weight DMAs (x first: LN1 tile 0 gates everything) ----
        x_tiles = []
        for mt in range(NMT):
            pool = xown_pool if mt < NMQ else xo_pool
            x_sb = pool.tile([128, E], bf16, name=f"x{mt}")
            nc.sync.dma_start(out=x_sb[:], in_=x_d[mt * 128:(mt + 1) * 128, :])
            x_tiles.append(x_sb)
        for j in range(NJE):
            nc.gpsimd.dma_start(out=wk_sb[:, j, :], in_=wk_d[j])
        for j in range(NJE):
            nc.gpsimd.dma_start(out=wq_sb[:, j, :], in_=wq_d[j])
        for j in range(NJE):
            nc.gpsimd.dma_start(out=wv_sb[:, j, :], in_=wv_d[j])
        # zero qT via a broadcast DMA on the idle ACT queue (a gpsimd memset
        # of 16KB/part would block the weight-DMA descriptors for ~14us)
        nc.scalar.dma_start(
            out=qT.rearrange("p h t -> p (h t)")[:],
            in_=z1_d[0:1, :].to_broadcast([128, H * TQ]))
        nc.gpsimd.memset(v_aug[:, :, :, :, HS:HS + 1], 1.0)  # ones col only

        # ---- emission helpers ----
        def emit_ln1_tile(mt):
            h_bf = hbp.tile([128, E], bf16)
            layernorm(stp, x_tiles[mt], h_bf, eps_sb)
            hg = hTp.tile([128, NJE, 128], bf16, name="hg")
            nc.sync.dma_start_transpose(out=hg[:], in_=h_bf[:])
            nc.vector.tensor_copy(out=h8[mt // 4][:, :, mt % 4, :], in_=hg[:])

        def emit_k_chunk(mf, g):
            pk = qkps.tile([128, 512], f32, name="ps_qkv")
            for j in range(0, NJE, 2):
                nc.tensor.matmul(pk[:], wk_sb[:, j:j + 2, mf * 128:(mf + 1) * 128],
                                 h8[g][:, j:j + 2, :, :], perf_mode=DR,
                                 start=(j == 0), stop=(j == NJE - 2))
            # K bias dropped: softmax is invariant to per-query score shifts
            nc.vector.tensor_copy(out=kT[:, mf, g * 512:(g + 1) * 512], in_=pk[:])

        def emit_q_chunk(mf, g):
            pq = qkps.tile([128, 512], f32, name="ps_qkv")
            for j in range(0, NJE, 2):
                nc.tensor.matmul(pq[:], wq_sb[:, j:j + 2, mf * 128:(mf + 1) * 128],
                                 h8[g][:, j:j + 2, :, :], perf_mode=DR,
                                 start=(j == 0), stop=(j == NJE - 2))
            h0, h1 = 2 * mf, 2 * mf + 1
            sl = slice(g * 512, (g + 1) * 512)
            nc.scalar.activation(out=qT[0:64, h0, sl], in_=pq[0:64, :],
                                 func=AF.Identity, bias=cq_sb[0:64, mf:mf + 1])
            nc.scalar.activation(out=qT[64:128, h1, sl], in_=pq[64:128, :],
                                 func=AF.Identity, bias=cq_sb[64:128, mf:mf + 1])

        def emit_v_chunk(st, vh):
            pv = qkps.tile([128, 512], f32, name="ps_qkv")
            for j in range(0, NJE, 2):
                nc.tensor.matmul(pv[:], h8[st // 4][:, j:j + 2, st % 4, :],
                                 wv_sb[:, j:j + 2, vh * 512:(vh + 1) * 512],
                                 perf_mode=DR, start=(j == 0), stop=(j == NJE - 2))
            # V bias passes through the softmax average exactly (weights sum
            # to 1), so it is folded into the proj bias host-side; this is a
            # plain copy, split ACT/DVE to balance the two engines.
            vdst = v_aug[:, st // 2, st % 2, vh * 8:(vh + 1) * 8, 0:HS]
            if vh == 0:
                nc.scalar.activation(
                    out=vdst, in_=pv.rearrange("p (h d) -> p h d", h=8),
                    func=AF.Identity)
            else:
                nc.vector.tensor_copy(
                    out=vdst, in_=pv.rearrange("p (h d) -> p h d", h=8))

        # ---- ramp: LN1 groups with K/Q/V woven in ----
        for g in range(4):
            for t in range(4):
                emit_ln1_tile(4 * g + t)
            emit_k_chunk(0, g)
            emit_k_chunk(1, g)
            if g < 2:
                emit_q_chunk(0, g)
                emit_q_chunk(1, g)
            for t in range(4):
                st = 4 * g + t
                emit_v_chunk(st, 0)
                emit_v_chunk(st, 1)
        ln_es.close()
        for j in range(NJE):
            nc.sync.dma_start(out=wo_sb[:, j, :], in_=wo_d[j])

        # ---- attention, with remaining K/Q chunks interleaved ----
        steps = [(h, st) for h in range(H) for st in range(NMT)]
        pairs = [(h, stp_) for h in range(H) for stp_ in range(NP)]
        work = []
        for mf in range(2, NJE):
            for g in range(4):
                work.append(lambda mf=mf, g=g: emit_k_chunk(mf, g))
            for g in range(2):
                work.append(lambda mf=mf, g=g: emit_q_chunk(mf, g))
        po_by_head = {}
        att_by_pair = {}

        def emit_scores(h, st):
            ps = aps.tile([128, TQ], f32, name="ps_sc")
            lhsT = kT[:, h // 2, st * 128:(st + 1) * 128]
            nc.tensor.matmul(ps[:, 0:512], lhsT, qT[:, h, 0:512],
                             start=True, stop=True)
            nc.tensor.matmul(ps[:, 512:1024], lhsT, qT[:, h, 512:1024],
                             start=True, stop=True)
            if st % 2 == 0:
                att_by_pair[(h, st // 2)] = atp.tile([128, 2, TQ], f8, name="att")
            at2 = att_by_pair[(h, st // 2)]
            nc.scalar.activation(out=at2[:, st % 2, :], in_=ps[:],
                                 func=AF.Exp, scale=float(HS) ** -0.5)

        def emit_av(h, stp_):
            # fp8 DoubleRow: contract both key tiles of the pair per matmul
            if stp_ == 0:
                po_by_head[h] = (ops.tile([HS + 1, 512], f32, name="ps_o"),
                                 ops.tile([HS + 1, 512], f32, name="ps_o"))
            po2 = po_by_head[h]
            at2 = att_by_pair.pop((h, stp_))
            vk = v_aug[:, stp_, :, h, :]
            for c in range(2):
                nc.tensor.matmul(po2[c][:], vk, at2[:, :, c * 512:(c + 1) * 512],
                                 perf_mode=DR,
                                 start=(stp_ == 0), stop=(stp_ == NP - 1))
            if stp_ == NP - 1:
                emit_head_finish(h, po2)

        def emit_head_finish(h, po2):
            # Drain PSUM immediately: store po*2^-9 (unnormalized, fp8) into
            # oT and the scaled denominator row to SBUF. The per-head DMA
            # round trip that reshapes the denominator to [128, TQ/128] for
            # the reciprocal (DVE divide is 8 cyc/elem, so partition-par)
            # then pipelines off the critical path; the in-place normalize
            # multiply lands on oT whenever rb arrives.
            p0 = (h % 2) * 64
            r1 = rp.tile([1, TQ], f32, name="rrow")
            ou = oup.tile([128, TQ], f8, name="ou")
            for c in range(2):
                sl = slice(c * 512, (c + 1) * 512)
                nc.vector.tensor_scalar_mul(out=r1[:, sl],
                                            in0=po2[c][HS:HS + 1, :],
                                            scalar1=2.0 ** -9)
                nc.vector.tensor_scalar_mul(out=ou[p0:p0 + 64, sl],
                                            in0=po2[c][0:HS, :],
                                            scalar1=2.0 ** -9)
            nc.sync.dma_start(out=rbounce[h:h + 1, :], in_=r1[:])
            rsq = rqp.tile([128, TQ // 128], f32, name="rsum")
            nc.sync.dma_start(
                out=rsq[:],
                in_=rbounce.rearrange("h (p c) -> h p c", p=128)[h])
            nc.vector.reciprocal(out=rsq[:], in_=rsq[:])
            nc.sync.dma_start(out=rbinv[h], in_=rsq[:])
            rb = rbp.tile([128, TQ], f32, name="rbc")
            nc.sync.dma_start(
                out=rb[:],
                in_=rbinv.rearrange("h p c -> h (p c)")[h:h + 1, :]
                .to_broadcast([128, TQ]))
            for c in range(2):
                sl = slice(c * 512, (c + 1) * 512)
                nc.vector.tensor_tensor(
                    out=oT[p0:p0 + 64, h // 2, sl],
                    in0=ou[p0:p0 + 64, sl], in1=rb[p0:p0 + 64, sl],
                    op=OP.mult)

        wi = 0
        for i, (h, st) in enumerate(steps):
            emit_scores(h, st)
            if i % 3 == 0 and wi < len(work):
                work[wi]()
                wi += 1
            if i >= 5 and i % 2 == 1:
                emit_av(*pairs[(i - 5) // 2])
        for j in range(len(pairs) - 2, len(pairs)):
            emit_av(*pairs[j])

        h8_es.close()
        at_es.close()
        wv_es.close()
        wkq_es.close()
        qkv_es.close()
        psA_es.close()

        # ---- proj + residual + LN2, pipelined into FFN1 (token halves) ----
        ffnT_es = ExitStack()
        ffnT = ffnT_es.enter_context(tc.tile_pool(name="ffnT", bufs=1, side="right")) \
            .tile([128, NJF, TQ], bf16)
        ffn_es = ExitStack()
        f1wp = ffn_es.enter_context(tc.tile_pool(name="f1w", bufs=4, side="left"))
        f1cp = ffn_es.enter_context(tc.tile_pool(name="f1c", bufs=1, side="left"))
        f1ps = ffn_es.enter_context(tc.tile_pool(name="f1ps", bufs=2, space="PSUM"))
        proj_es = ExitStack()
        stp2 = proj_es.enter_context(tc.tile_pool(name="ln2s", bufs=8, side="left"))
        hbp2 = proj_es.enter_context(tc.tile_pool(name="ln2h", bufs=3, side="left"))
        pps = proj_es.enter_context(tc.tile_pool(name="proj_ps", bufs=4, space="PSUM"))
        b1_sb = f1cp.tile([128, NJF], f32)
        nc.gpsimd.dma_start(out=b1_sb[:], in_=b1c_d[:])

        def emit_proj(mt):
            pa = pps.tile([128, 512], f32, name="ps_pr")
            pb = pps.tile([128, 512], f32, name="ps_pr")
            for j in range(0, NJE, 2):
                lhsT = oT[:, j:j + 2, mt * 128:(mt + 1) * 128]
                nc.tensor.matmul(pa[:], lhsT, wo_sb[:, j:j + 2, 0:512],
                                 perf_mode=DR, start=(j == 0), stop=(j == NJE - 2))
                nc.tensor.matmul(pb[:], lhsT, wo_sb[:, j:j + 2, 512:1024],
                                 perf_mode=DR, start=(j == 0), stop=(j == NJE - 2))
            # xr = (proj + bo) + x   (x from the bf16 input tiles)
            for c, pc in ((0, pa), (1, pb)):
                sl = slice(c * 512, (c + 1) * 512)
                nc.vector.tensor_tensor(out=xr_t[mt][:, sl], in0=pc[:],
                                        in1=bob_sb[:, sl], op=OP.add)
                nc.vector.tensor_tensor(out=xr_t[mt][:, sl], in0=xr_t[mt][:, sl],
                                        in1=x_tiles[mt][:, sl], op=OP.add)
            h_bf = hbp2.tile([128, E], bf16)
            layernorm(stp2, xr_t[mt][:], h_bf, eps_sb)
            nc.sync.dma_start_transpose(out=h2T[:, mt, :, :], in_=h_bf[:])

        def emit_ffn1(mf, half):
            w1_sb = f1wp.tile([128, NJE, 128], bf16, name="w1t")
            eng = nc.gpsimd if mf % 2 == 0 else nc.scalar
            eng.dma_start(out=w1_sb[:],
                          in_=w1_d[mf].rearrange("p (j c) -> p j c", j=NJE))
            pf = f1ps.tile([128, 512], f32, name="ps_f1")
            for j in range(NJE):
                nc.tensor.matmul(pf[:], w1_sb[:, j, :],
                                 h2T[:, 4 * half:4 * half + 4, j, :],
                                 start=(j == 0), stop=(j == NJE - 1))
            nc.scalar.activation(out=ffnT[:, mf, half * 512:(half + 1) * 512],
                                 in_=pf[:], func=AF.Relu,
                                 bias=b1_sb[:, mf:mf + 1])

        for mt in range(4):
            emit_proj(mt)
        # FFN1 half 0 interleaved with proj/LN2 of the second token half
        for mf in range(NJF):
            emit_ffn1(mf, 0)
            if mf < 4:
                emit_proj(4 + mf)
        proj_es.close()

        # ---- FFN2 (nbh0, mt0-3) interleaved with FFN1 half 1 ----
        f2_es = ExitStack()
        f2wp = f2_es.enter_context(tc.tile_pool(name="f2w", bufs=8, side="left"))
        f2cp = f2_es.enter_context(tc.tile_pool(name="f2c", bufs=1, side="left"))
        f2op = f2_es.enter_context(tc.tile_pool(name="f2o", bufs=3, side="left"))
        f2ps = f2_es.enter_context(tc.tile_pool(name="f2ps", bufs=4, space="PSUM"))
        b2_sb = f2cp.tile([128, E], f32)
        nc.gpsimd.dma_start(out=b2_sb[:], in_=b2b_d[:])

        def emit_f2_out(mt, nbh, psum):
            o_sb = f2op.tile([128, 512], f32, name="osb")
            nc.vector.tensor_tensor(out=o_sb[:], in0=psum[:],
                                    in1=xr_t[mt][:, nbh * 512:(nbh + 1) * 512],
                                    op=OP.add)
            nc.vector.tensor_tensor(out=o_sb[:], in0=o_sb[:],
                                    in1=b2_sb[:, nbh * 512:(nbh + 1) * 512],
                                    op=OP.add)
            nc.sync.dma_start(
                out=out_d[mt * 128:(mt + 1) * 128, nbh * 512:(nbh + 1) * 512],
                in_=o_sb[:])

        psA = [f2ps.tile([128, 512], f32, name="ps_f2") for _ in range(4)]
        for k in range(NJF):
            w2_sb = f2wp.tile([128, 512], bf16, name="w2t")
            (nc.gpsimd if k % 2 == 0 else nc.scalar).dma_start(
                out=w2_sb[:], in_=w2_d[k][:, 0:512])
            for mt in range(4):
                nc.tensor.matmul(psA[mt][:], ffnT[:, k, mt * 128:(mt + 1) * 128],
                                 w2_sb[:], start=(k == 0), stop=(k == NJF - 1))
            emit_ffn1(k, 1)
        for mt in range(4):
            emit_f2_out(mt, 0, psA[mt])

        # remaining three quadrants: (nbh1, mt0-3), then mt4-7 both halves
        psB = [f2ps.tile([128, 512], f32, name="ps_f2") for _ in range(4)]
        for k in range(NJF):
            w2_sb = f2wp.tile([128, 512], bf16, name="w2t")
            (nc.gpsimd if k % 2 == 0 else nc.scalar).dma_start(
                out=w2_sb[:], in_=w2_d[k][:, 512:1024])
            for mt in range(4):
                nc.tensor.matmul(psB[mt][:], ffnT[:, k, mt * 128:(mt + 1) * 128],
                                 w2_sb[:], start=(k == 0), stop=(k == NJF - 1))
        for mt in range(4):
            emit_f2_out(mt, 1, psB[mt])

        for nbh in range(2):
            psums = [f2ps.tile([128, 512], f32, name="ps_f2") for _ in range(4)]
            for k in range(NJF):
                w2_sb = f2wp.tile([128, 512], bf16, name="w2t")
                (nc.gpsimd if k % 2 == 0 else nc.scalar).dma_start(
                    out=w2_sb[:], in_=w2_d[k][:, nbh * 512:(nbh + 1) * 512])
                for mt in range(4):
                    nc.tensor.matmul(psums[mt][:],
                                     ffnT[:, k, (4 + mt) * 128:(5 + mt) * 128],
                                     w2_sb[:], start=(k == 0), stop=(k == NJF - 1))
            for mt in range(4):
                emit_f2_out(4 + mt, nbh, psums[mt])

        f2_es.close()
        ffn_es.close()
        ffnT_es.close()
        h2T_es.close()
        wop_es.close()
        oT_es.close()
        xo_es.close()
        top.close()

    nc.compile()
    return nc


def _prep_weights(ln1_g, ln1_b, Wq, Wk, Wv, Wo, bo, ln2_g, ln2_b, W1, b1, W2, b2):
    f64 = np.float64
    g1 = np.asarray(ln1_g, f64)
    b1ln = np.asarray(ln1_b, f64)
    g2 = np.asarray(ln2_g, f64)
    b2ln = np.asarray(ln2_b, f64)

    def flat_qkv(W):
        return np.asarray(W, f64).transpose(1, 0, 2).reshape(E, H * HS)

    Wqf, Wkf, Wvf = flat_qkv(Wq), flat_qkv(Wk), flat_qkv(Wv)
    out = {}
    out["wq"] = np.ascontiguousarray((g1[:, None] * Wqf).reshape(NJE, 128, E).astype(F8))
    out["wk"] = np.ascontiguousarray((g1[:, None] * Wkf).reshape(NJE, 128, E).astype(F8))
    out["wv"] = np.ascontiguousarray((g1[:, None] * Wvf).reshape(NJE, 128, E).astype(F8))
    cq = (b1ln @ Wqf).astype(np.float32)
    cv = b1ln @ Wvf
    out["cq"] = np.ascontiguousarray(cq.reshape(NJE, 128).T)
    # V bias rides through the softmax average (weights sum to 1), so it
    # folds into the attention-proj bias: bob = bo + cv @ Wo
    bo_eff = (np.asarray(bo, f64) + cv @ np.asarray(Wo, f64)).astype(np.float32)
    out["bob"] = np.ascontiguousarray(np.broadcast_to(bo_eff, (128, E)))
    out["z1"] = np.zeros((1, H * TQ), F8)
    out["wo"] = np.ascontiguousarray(np.asarray(Wo, f64).reshape(NJE, 128, E).astype(F8))
    W1p = g2[:, None] * np.asarray(W1, f64)
    b1p = (np.asarray(b1, f64) + b2ln @ np.asarray(W1, f64)).astype(np.float32)
    out["w1"] = np.ascontiguousarray(
        W1p.reshape(NJE, 128, NJF, 128).transpose(2, 1, 0, 3).reshape(NJF, 128, E).astype(BF))
    out["b1c"] = np.ascontiguousarray(b1p.reshape(NJF, 128).T)
    out["w2"] = np.ascontiguousarray(np.asarray(W2, f64).reshape(NJF, 128, E).astype(BF))
    out["b2b"] = np.ascontiguousarray(
        np.broadcast_to(np.asarray(b2, np.float32), (128, E)))
    return out


def kernel(x, ln1_g, ln1_b, Wq, Wk, Wv, Wo, bo, ln2_g, ln2_b, W1, b1, W2, b2):
    global LAST_RESULTS
    from concourse.bass_utils import run_bass_kernel_spmd

    if "nc" not in _CACHE:
        _CACHE["nc"] = _build()
    nc = _CACHE["nc"]

    wmap = _prep_weights(ln1_g, ln1_b, Wq, Wk, Wv, Wo, bo,
                         ln2_g, ln2_b, W1, b1, W2, b2)
    x = np.asarray(x, np.float32)

    in_maps = []
    for c in range(NCORES):
        b, half = c // 2, c % 2
        xb = x[b]
        x_roll = np.ascontiguousarray(
            np.concatenate([xb[half * TQ:], xb[:half * TQ]], axis=0))
        m = dict(wmap)
        m["x"] = x_roll.astype(BF)
        in_maps.append(m)

    res = run_bass_kernel_spmd(nc, in_maps, list(range(NCORES)), trace=TRACE)
    LAST_RESULTS = res

    out = np.empty((B, T, E), np.float32)
    for c in range(NCORES):
        b, half = c // 2, c % 2
        out[b, half * TQ:(half + 1) * TQ] = res.results[c]["out"]
    return out
